# revision 1
# baseline (speedup 1.0000x reference)
"""Trainium2 Bass kernel for nn_MultiHeadAttention (B=2, S=2048, D=1024, H=16, causal).

Sharding across 8 NeuronCores (single SPMD program):
  - Core c owns batch b=c//4 and two 256-token query chunks {p, 7-p} (p=c%4)
    of that batch; the pairing balances causal attention work (every core
    covers 18 key-blocks of true work).
  - Phase 1: each core projects Q/K/V for its 512 tokens at full width.
    The 1/sqrt(64) score scale is folded into Wk/bk on the host.
  - Two AllGathers (replica groups [[0-3],[4-7]], i.e. per batch) publish
    K^T and V so that addresses are identical on every core.
  - Phase 2: streaming softmax in transposed layout scoresT[k, q] (no
    on-chip transposes); the softmax denominator falls out of an extra
    ones-column in the V operand of the attn@V matmul.  Causal masking and
    the per-core staircase use host-precomputed additive mask tiles, with
    uniform loop extents (8 blocks for the low chunk, 16 for the high one).
  - Phase 3: output projection for the core's own tokens only (row-parallel
    over tokens => no reduction); the host re-assembles the full output.
"""
import numpy as np

import concourse.bass as bass
import concourse.bacc as bacc
import concourse.mybir as mybir
import concourse.tile as tile
from concourse.bass_utils import run_bass_kernel_spmd
from concourse.tile_rust import add_dep_helper

B, S, D, H, HD = 2, 2048, 1024, 16, 64
NC = 8
P = 128
NEG = -1e10
F32 = mybir.dt.float32

# compute dtype for matmul-feeding tensors: float32r runs the PE at 4x the
# fp32 rate (1 cyc/row at N>=256); the BIR verifier requires the whole
# producer chain of a float32r matmul operand to be declared float32r.
CDT = mybir.dt.float32r

TRACE = False        # set True (e.g. from test.py) to capture an NTFF profile
LAST_RESULT = None   # BassKernelResults of the most recent kernel() call


def _mm(ap):
    return ap


def sel_tokens(p):
    return list(range(256 * p, 256 * p + 256)) + list(
        range(256 * (7 - p), 256 * (7 - p) + 256)
    )


def _kblk(j):
    """Original 128-token key block j -> (rank-in-group, column offset)."""
    q = j // 2
    rr = q if q <= 3 else 7 - q
    off = (0 if q <= 3 else 256) + 128 * (j % 2)
    return rr, off


def _emit(causal: bool, repeat: int = 1):
    nc = bacc.Bacc(trn_type="TRN2", num_devices=NC)
    ident = mybir.ActivationFunctionType.Identity
    fexp = mybir.ActivationFunctionType.Exp

    xT = nc.dram_tensor("xT", [D, 512], CDT, kind="ExternalInput")
    wqT = nc.dram_tensor("wqT", [D, D], CDT, kind="ExternalInput")
    wkT = nc.dram_tensor("wkT", [D, D], CDT, kind="ExternalInput")
    wvT = nc.dram_tensor("wvT", [D, D], CDT, kind="ExternalInput")
    woT = nc.dram_tensor("woT", [D, D], CDT, kind="ExternalInput")
    bq_d = nc.dram_tensor("bq", [P, 8], F32, kind="ExternalInput")
    bk_d = nc.dram_tensor("bk", [P, 8], F32, kind="ExternalInput")
    bv_d = nc.dram_tensor("bv", [1, D], CDT, kind="ExternalInput")
    bo_d = nc.dram_tensor("bo", [P, 8], F32, kind="ExternalInput")
    if causal:
        cmb_d = nc.dram_tensor("cmb", [P, 16, 256], F32, kind="ExternalInput")
    outT = nc.dram_tensor("outT", [D, 512], F32, kind="ExternalOutput")

    kt_loc = nc.dram_tensor("kt_loc", [D, 512], CDT)
    v_loc = nc.dram_tensor("v_loc", [512, D], CDT)
    kt_all = nc.dram_tensor("kt_all", [4 * D, 512], CDT)
    v_all = nc.dram_tensor("v_all", [4 * 512, D], CDT)

    with tile.TileContext(nc) as tc, \
         tc.tile_pool(name="const", bufs=1) as const, \
         tc.tile_pool(name="w", bufs=1) as wpool, \
         tc.tile_pool(name="big", bufs=1) as big, \
         tc.tile_pool(name="io", bufs=3) as io, \
         tc.tile_pool(name="kv", bufs=6) as kv, \
         tc.tile_pool(name="ex", bufs=4) as ex, \
         tc.tile_pool(name="sm", bufs=2) as sm, \
         tc.tile_pool(name="ps_big", bufs=2, space="PSUM") as ps_big, \
         tc.tile_pool(name="ps_sc", bufs=3, space="PSUM") as ps_sc, \
         tc.tile_pool(name="ps_ctx", bufs=2, space="PSUM") as ps_ctx, \
         tc.tile_pool(name="ps_rep", bufs=1, space="PSUM") as ps_rep:

        # ---------- constants ----------
        ones_f = const.tile([P, P], F32)
        nc.gpsimd.memset(ones_f[:], 1.0)
        ones = const.tile([P, P], CDT)
        nc.vector.tensor_copy(ones[:], ones_f[:])
        bq_sb = const.tile([P, 8], F32)
        nc.sync.dma_start(bq_sb[:], bq_d[:])
        bk_sb = const.tile([P, 8], F32)
        nc.sync.dma_start(bk_sb[:], bk_d[:])
        bv_sb = const.tile([1, D], CDT)
        nc.sync.dma_start(bv_sb[:], bv_d[:])
        bo_sb = const.tile([P, 8], F32)
        nc.sync.dma_start(bo_sb[:], bo_d[:])
        if causal:
            cmb_sb = big.tile([P, 16, 256], F32)
            nc.sync.dma_start(cmb_sb[:], cmb_d[:])

        for _rep in range(repeat):
            # ---------- phase 1: projections for this core's 512 tokens ----------
            xt_sb = big.tile([P, 8, 512], CDT)
            xr = xT.rearrange("(o p) t -> p o t", p=P)
            for _kt in range(8):
                nc.sync.dma_start(xt_sb[:, _kt, :], xr[:, _kt, :])
            qt_sb = big.tile([P, 8, 512], CDT)

            def proj_qk(w_dram, bias_sb, to_dram):
                w_sb = wpool.tile([P, 8, D], CDT, tag="w")
                nc.sync.dma_start(w_sb[:], w_dram.rearrange("(o p) t -> p o t", p=P))
                for dt in range(8):
                    pt = ps_big.tile([P, 512], F32)
                    for kt in range(8):
                        nc.tensor.matmul(
                            pt[:], _mm(w_sb[:, kt, 128 * dt:128 * dt + 128]),
                            _mm(xt_sb[:, kt, :]), start=(kt == 0), stop=(kt == 7))
                    if to_dram is None:
                        nc.scalar.activation(qt_sb[:, dt, :], pt[:], ident,
                                             bias=bias_sb[:, dt:dt + 1])
                    else:
                        t = io.tile([P, 512], CDT, tag="io")
                        nc.scalar.activation(t[:], pt[:], ident,
                                             bias=bias_sb[:, dt:dt + 1])
                        nc.sync.dma_start(
                            to_dram.rearrange("(o p) t -> p o t", p=P)[:, dt, :],
                            t[:])

            rg = [[0, 1, 2, 3], [4, 5, 6, 7]]

            proj_qk(wkT, bk_sb, kt_loc)
            # kick off the K AllGather while V/Q projections still run
            cc_k = nc.gpsimd.collective_compute(
                "AllGather", mybir.AluOpType.bypass, replica_groups=rg,
                ins=[kt_loc[:]], outs=[kt_all[:]])

            wv_sb = wpool.tile([P, 8, D], CDT, tag="w")
            wvr = wvT.rearrange("(o p) t -> p o t", p=P)
            for _kt in range(8):
                nc.sync.dma_start(wv_sb[:, _kt, :], wvr[:, _kt, :])
            for st in range(4):
                for hf in range(2):
                    pt = ps_big.tile([P, 512], F32)
                    for kt in range(8):
                        nc.tensor.matmul(
                            pt[:], _mm(xt_sb[:, kt, 128 * st:128 * st + 128]),
                            _mm(wv_sb[:, kt, 512 * hf:512 * hf + 512]),
                            start=(kt == 0), stop=False)
                    nc.tensor.matmul(
                        pt[:], _mm(ones[0:1, 0:P]),
                        _mm(bv_sb[0:1, 512 * hf:512 * hf + 512]),
                        start=False, stop=True)
                    t = io.tile([P, 512], CDT, tag="io")
                    nc.scalar.copy(t[:], pt[:])
                    nc.sync.dma_start(
                        v_loc[128 * st:128 * st + 128, 512 * hf:512 * hf + 512],
                        t[:])
            cc_v = nc.gpsimd.collective_compute(
                "AllGather", mybir.AluOpType.bypass, replica_groups=rg,
                ins=[v_loc[:]], outs=[v_all[:]])

            proj_qk(wqT, bq_sb, None)
            wo_sb = wpool.tile([P, 8, D], CDT, tag="w")
            nc.sync.dma_start(wo_sb[:],
                              woT.rearrange("(o p) t -> p o t", p=P))

            # ---------- phase 2: attention ----------
            # Single merged pass: key-blocks 0..7 are valid for BOTH q-chunks
            # (role 1's true extent is always >= 10), so supers 0..1 process
            # them once at N=512 across both chunks; supers 2..3 (blocks
            # 8..15) touch only the high chunk (cols 256:512) when causal.
            # kv blocks are loaded once per super and shared by all heads;
            # per-head ctx accumulates in PSUM within a super and in SBUF
            # (acc, row 64 = softmax denominator) across supers.
            ctx_sb = big.tile([P, 8, 512], CDT)
            acc = big.tile([P, H, 512], F32)
            for sj in range(4):
                wid = 512 if (not causal or sj < 2) else 256
                qoff = 0 if (not causal or sj < 2) else 256
                kts, vas = [], []
                for jj in range(4):
                    j = 4 * sj + jj
                    rr, off = _kblk(j)
                    kt_t = kv.tile([P, 8, 128], CDT, tag="kt")
                    d1 = nc.sync.dma_start(
                        kt_t[:],
                        kt_all.rearrange("(r o p) t -> p r o t", p=P, o=8)
                        [:, rr, :, off:off + 128])
                    add_dep_helper(d1.ins, cc_k.ins, reason="read after AG-K")
                    va = kv.tile([P, H, 66], CDT, tag="va")
                    d2 = nc.sync.dma_start(
                        va[:, :, 1:65],
                        v_all[512 * rr + off:512 * rr + off + 128, :]
                        .rearrange("p (h d) -> p h d", h=H))
                    add_dep_helper(d2.ins, cc_v.ins, reason="read after AG-V")
                    nc.vector.tensor_copy(va[:, :, 65:66],
                                          ones[:, 0:H, None])
                    kts.append(kt_t)
                    vas.append(va)
                for h in range(H):
                    hb = 64 * (h % 2)
                    ctx_ps = ps_ctx.tile([P, 512], F32)
                    for jj in range(4):
                        j = 4 * sj + jj
                        sc = ps_sc.tile([P, 512], F32)
                        nc.tensor.matmul(
                            sc[:, 0:wid],
                            _mm(kts[jj][hb:hb + 64, h // 2, :]),
                            _mm(qt_sb[hb:hb + 64, h // 2,
                                      qoff:qoff + wid]),
                            start=True, stop=True)
                        if causal:
                            # mask the low (sj<2: role-0) / high (sj>=2:
                            # role-1) chunk's 256 columns of this block
                            nc.vector.tensor_tensor(
                                sc[:, 0:256], sc[:, 0:256], cmb_sb[:, j, :],
                                mybir.AluOpType.add)
                        et = ex.tile([P, 512], CDT, tag="exp")
                        nc.scalar.activation(et[:, 0:wid], sc[:, 0:wid], fexp)
                        # ctx rows 0:64, softmax-denominator row at 64
                        nc.tensor.matmul(ctx_ps[0:65, 0:wid],
                                         _mm(vas[jj][:, h, 1:66]),
                                         _mm(et[:, 0:wid]), start=(jj == 0),
                                         stop=(jj == 3))
                    if sj == 0:
                        nc.vector.tensor_copy(acc[0:65, h, :],
                                              ctx_ps[0:65, :])
                    else:
                        nc.vector.tensor_tensor(
                            acc[0:65, h, qoff:qoff + wid],
                            ctx_ps[0:65, 0:wid],
                            acc[0:65, h, qoff:qoff + wid],
                            mybir.AluOpType.add)
            # normalize + output-project one q-half (cols off:off+w).
            # When causal, the low half is final after supers 0..1, so its
            # tail + projection overlap supers 2..3.
            def norm_and_proj(off, w, wo_sb):
                for h in range(H):
                    recip_sb = sm.tile([P, 512], CDT, tag="recip")
                    with nc.allow_low_precision(
                            reason="softmax denom in f32r"):
                        nc.vector.reciprocal(recip_sb[64:65, off:off + w],
                                             acc[64:65, h, off:off + w])
                    rep_ps = ps_rep.tile([P, 512], F32)
                    nc.tensor.matmul(rep_ps[0:64, 0:w],
                                     _mm(ones[64:65, 0:64]),
                                     _mm(recip_sb[64:65, off:off + w]),
                                     start=True, stop=True)
                    rep_sb = sm.tile([P, 512], F32, tag="rep")
                    nc.scalar.copy(rep_sb[0:64, 0:w], rep_ps[0:64, 0:w])
                    if h % 2 == 0:
                        nc.vector.tensor_tensor(
                            ctx_sb[0:64, h // 2, off:off + w],
                            acc[0:64, h, off:off + w],
                            rep_sb[0:64, 0:w], mybir.AluOpType.mult)
                    else:
                        tmp = sm.tile([P, 512], CDT, tag="ctxtmp")
                        nc.vector.tensor_tensor(
                            tmp[0:64, 0:w], acc[0:64, h, off:off + w],
                            rep_sb[0:64, 0:w], mybir.AluOpType.mult)
                        nc.sync.dma_start(
                            ctx_sb[64:128, h // 2, off:off + w],
                            tmp[0:64, 0:w])
                for m in range(8):
                    pt = ps_big.tile([P, 512], F32)
                    for kt in range(8):
                        nc.tensor.matmul(
                            pt[:, 0:w],
                            _mm(wo_sb[:, kt, 128 * m:128 * m + 128]),
                            _mm(ctx_sb[:, kt, off:off + w]),
                            start=(kt == 0), stop=(kt == 7))
                    t = io.tile([P, 512], F32, tag="io")
                    nc.scalar.activation(t[:, 0:w], pt[:, 0:w], ident,
                                         bias=bo_sb[:, m:m + 1])
                    nc.sync.dma_start(
                        outT.rearrange("(o p) t -> p o t", p=P)
                        [:, m, off:off + w], t[:, 0:w])

            if causal:
                norm_and_proj(0, 256, wo_sb)
                norm_and_proj(256, 256, wo_sb)
            else:
                norm_and_proj(0, 512, wo_sb)

    nc.compile()
    return nc


_CACHE = {}


def _get_nc(causal: bool, repeat: int = 1):
    key = (causal, repeat)
    if key not in _CACHE:
        _CACHE[key] = _emit(causal, repeat)
    return _CACHE[key]


def _mask_tiles(p):
    """Per-core additive mask [128, 16, 256] for causal staircase."""
    k = np.arange(128)[:, None]
    c = np.arange(256)[None, :]
    m1 = np.where(c - k >= 0, 0.0, NEG).astype(np.float32)
    m2 = np.where(c - 128 - k >= 0, 0.0, NEG).astype(np.float32)
    cmb = np.zeros((128, 16, 256), dtype=np.float32)
    # slots 0..7: role 0 (chunk p), true extent 2p+2
    for j in range(8):
        if j == 2 * p:
            cmb[:, j, :] = m1
        elif j == 2 * p + 1:
            cmb[:, j, :] = m2
        elif j > 2 * p + 1:
            cmb[:, j, :] = NEG
    # slots 8..15: role 1 (chunk 7-p), true extent 16-2p; blocks 0..7 unmasked
    for j in range(8, 16):
        if j == 14 - 2 * p:
            cmb[:, j, :] = m1
        elif j == 15 - 2 * p:
            cmb[:, j, :] = m2
        elif j > 15 - 2 * p:
            cmb[:, j, :] = NEG
    return cmb


def kernel(**inputs):
    x = np.ascontiguousarray(np.asarray(inputs["x"], dtype=np.float32))
    Wq = np.asarray(inputs["Wq"], dtype=np.float32)
    bq = np.asarray(inputs["bq"], dtype=np.float32)
    Wk = np.asarray(inputs["Wk"], dtype=np.float32)
    bk = np.asarray(inputs["bk"], dtype=np.float32)
    Wv = np.asarray(inputs["Wv"], dtype=np.float32)
    bv = np.asarray(inputs["bv"], dtype=np.float32)
    Wo = np.asarray(inputs["Wo"], dtype=np.float32)
    bo = np.asarray(inputs["bo"], dtype=np.float32)
    causal = bool(int(np.asarray(inputs["enable_causal"])))

    scale = np.float32(1.0 / np.sqrt(HD))
    wqT = np.ascontiguousarray(Wq.T)
    wkT = np.ascontiguousarray((Wk * scale).T)
    wvT = np.ascontiguousarray(Wv.T)
    woT = np.ascontiguousarray(Wo.T)
    bqt = np.ascontiguousarray(bq.reshape(8, P).T)
    bkt = np.ascontiguousarray((bk * scale).reshape(8, P).T)
    bvr = np.ascontiguousarray(bv.reshape(1, D))
    bot = np.ascontiguousarray(bo.reshape(8, P).T)

    nc = _get_nc(causal)
    in_maps = []
    for c in range(NC):
        b, p = divmod(c, 4)
        sel = sel_tokens(p)
        xTc = np.ascontiguousarray(x[b][sel, :].T)
        m = {"xT": xTc, "wqT": wqT, "wkT": wkT, "wvT": wvT, "woT": woT,
             "bq": bqt, "bk": bkt, "bv": bvr, "bo": bot}
        if causal:
            m["cmb"] = _mask_tiles(p)
        in_maps.append(m)

    global LAST_RESULT
    res = run_bass_kernel_spmd(nc, in_maps, list(range(NC)), trace=TRACE)
    LAST_RESULT = res
    out = np.empty((B, S, D), dtype=np.float32)
    for c in range(NC):
        b, p = divmod(c, 4)
        sel = sel_tokens(p)
        out[b, sel, :] = res.results[c]["outT"].T
    return out



# revision 4
# speedup vs baseline: 1.8576x; 1.8576x over previous
"""Trainium2 Bass kernel for nn_MultiHeadAttention (B=2, S=2048, D=1024, H=16, causal).

Sharding across 8 NeuronCores (single SPMD program):
  - Core c owns batch b=c//4 and two 256-token query chunks {p, 7-p} (p=c%4);
    the pairing balances causal attention work.
  - Everything on-chip is bf16 (PSUM accumulation stays fp32): halves HBM +
    collective bytes vs fp32 and enables fast weight loads (FWL) on the PE.
  - Phase 1: project K^T, V (with softmax scale folded into Wk/bk on the
    host), publish both with ONE AllGather (replica groups [[0-3],[4-7]]);
    Q projection + Wo load overlap the collective.  All biases are applied
    with rank-1 matmuls into PSUM (no scalar-engine bias pass).
  - Phase 2: K^T and V for the whole batch live in SBUF.  Heads are
    processed in pairs (feature block = 128 partitions); per key block the
    two heads' score matmuls are row-tiled (partitions 0:64 / 64:128) into
    one 2-bank PSUM tile so they run concurrently, one wide exp covers both,
    causal masking is a single multiplicative bf16 DVE op on the (host
    per-core) staircase, and ctx accumulates in PSUM across ALL 16 key
    blocks (65th stationary column = softmax denominator).  The softmax
    reciprocal is exp(-ln(d)) on the scalar engine (DVE reciprocal is
    8 cyc/elem and was 67us in the fp32 baseline).
  - Phase 3: output projection for the core's own tokens (row-parallel over
    tokens => no reduction); host re-assembles the full output.
"""
import numpy as np
import ml_dtypes

import concourse.bass as bass
import concourse.bacc as bacc
import concourse.mybir as mybir
import concourse.tile as tile
from concourse.bass_utils import run_bass_kernel_spmd
from concourse.tile_rust import add_dep_helper

B, S, D, H, HD = 2, 2048, 1024, 16, 64
NC = 8
P = 128
F32 = mybir.dt.float32
BF = mybir.dt.bfloat16
NPBF = ml_dtypes.bfloat16

KT_N = D * 512           # K^T shard elems  [1024, 512]
V_N = 512 * 16 * 65      # V shard elems    [512 tok, 16 heads, 64+1]
KV_N = KT_N + V_N

TRACE = False        # set True (e.g. from test.py) to capture an NTFF profile
LAST_RESULT = None   # BassKernelResults of the most recent kernel() call


def sel_tokens(p):
    return list(range(256 * p, 256 * p + 256)) + list(
        range(256 * (7 - p), 256 * (7 - p) + 256)
    )


def _kblk(j):
    """Original 128-token key block j -> (rank-in-group, column offset)."""
    q = j // 2
    rr = q if q <= 3 else 7 - q
    off = (0 if q <= 3 else 256) + 128 * (j % 2)
    return rr, off


def _emit(causal: bool, repeat: int = 1):
    nc = bacc.Bacc(trn_type="TRN2", num_devices=NC)
    fexp = mybir.ActivationFunctionType.Exp
    fln = mybir.ActivationFunctionType.Ln

    xT = nc.dram_tensor("xT", [D, 512], BF, kind="ExternalInput")
    wqT = nc.dram_tensor("wqT", [D, D], BF, kind="ExternalInput")
    wkT = nc.dram_tensor("wkT", [D, D], BF, kind="ExternalInput")
    wvT = nc.dram_tensor("wvT", [D, D], BF, kind="ExternalInput")
    woT = nc.dram_tensor("woT", [D, D], BF, kind="ExternalInput")
    bq_d = nc.dram_tensor("bq", [1, D], BF, kind="ExternalInput")
    bk_d = nc.dram_tensor("bk", [1, D], BF, kind="ExternalInput")
    bv_d = nc.dram_tensor("bv", [1, D], BF, kind="ExternalInput")
    bo_d = nc.dram_tensor("bo", [1, D], BF, kind="ExternalInput")
    if causal:
        cmb_d = nc.dram_tensor("cmb", [P, 16, 2, 256], BF, kind="ExternalInput")
    outT = nc.dram_tensor("outT", [D, 512], F32, kind="ExternalOutput")

    kv_loc = nc.dram_tensor("kv_loc", [KV_N], BF)
    kv_all = nc.dram_tensor("kv_all", [4, KV_N], BF)

    with tile.TileContext(nc) as tc, \
         tc.tile_pool(name="const", bufs=1) as const, \
         tc.tile_pool(name="w", bufs=1) as wpool, \
         tc.tile_pool(name="big", bufs=1) as big, \
         tc.tile_pool(name="kv", bufs=1) as kvp, \
         tc.tile_pool(name="io", bufs=3) as io, \
         tc.tile_pool(name="vio", bufs=2) as vio, \
         tc.tile_pool(name="oio", bufs=2) as oio, \
         tc.tile_pool(name="ex", bufs=3) as ex, \
         tc.tile_pool(name="sm", bufs=2) as sm, \
         tc.tile_pool(name="ps_sc", bufs=2, space="PSUM") as ps_sc, \
         tc.tile_pool(name="ps_ctx", bufs=2, space="PSUM") as ps_ctx, \
         tc.tile_pool(name="ps_w", bufs=2, space="PSUM") as ps_w:

        # ---------- constants ----------
        ones = const.tile([P, 512], BF)
        nc.gpsimd.memset(ones[:], 1.0)
        bq_sb = const.tile([1, D], BF)
        nc.sync.dma_start(bq_sb[:], bq_d[:])
        bk_sb = const.tile([1, D], BF)
        nc.sync.dma_start(bk_sb[:], bk_d[:])
        bv_sb = const.tile([1, D], BF)
        nc.sync.dma_start(bv_sb[:], bv_d[:])
        bo_sb = const.tile([1, D], BF)
        nc.sync.dma_start(bo_sb[:], bo_d[:])
        if causal:
            cmb_sb = big.tile([P, 16, 2, 256], BF)
            nc.sync.dma_start(cmb_sb[:], cmb_d[:])

        rg = [[0, 1, 2, 3], [4, 5, 6, 7]]
        kt_ap = kv_loc[0:KT_N].rearrange("(o p t) -> p o t", o=8, p=P, t=512)
        v_ap = kv_loc[KT_N:KV_N].rearrange("(a p h c) -> p a h c",
                                           a=4, p=P, h=16, c=65)

        for _rep in range(repeat):
            # ---------- phase 1: projections for this core's 512 tokens ----
            xt_sb = big.tile([P, 8, 512], BF)
            xr = xT.rearrange("(o p) t -> p o t", p=P)
            for _kt in range(8):
                nc.sync.dma_start(xt_sb[:, _kt, :], xr[:, _kt, :])
            qt_sb = big.tile([P, 8, 512], BF)

            def load_w(w_dram):
                w_sb = wpool.tile([P, 8, D], BF, tag="w")
                wr = w_dram.rearrange("(o p) t -> p o t", p=P)
                for _kt in range(8):
                    nc.sync.dma_start(w_sb[:, _kt, :], wr[:, _kt, :])
                return w_sb

            def proj_T(w_sb, bias_sb, sink):
                # out[feat, tok]: per-partition bias via rank-1 matmul
                for dt in range(8):
                    pt = ps_w.tile([P, 512], F32, tag="psw")
                    for kt in range(8):
                        nc.tensor.matmul(
                            pt[:], w_sb[:, kt, 128 * dt:128 * dt + 128],
                            xt_sb[:, kt, :], start=(kt == 0), stop=False)
                    nc.tensor.matmul(
                        pt[:], bias_sb[0:1, 128 * dt:128 * dt + 128],
                        ones[0:1, 0:512], start=False, stop=True)
                    sink(dt, pt)

            # K^T -> kv_loc
            wk_sb = load_w(wkT)

            def k_sink(dt, pt):
                t = io.tile([P, 512], BF, tag="io")
                nc.vector.tensor_copy(t[:], pt[:])
                nc.sync.dma_start(kt_ap[:, dt, :], t[:])
            proj_T(wk_sb, bk_sb, k_sink)

            # V -> kv_loc ([tok, head, 64] + ones column)
            wv_sb = load_w(wvT)
            for st in range(4):
                vt = vio.tile([P, 16, 65], BF, tag="vio")
                for hf in range(2):
                    pt = ps_w.tile([P, 512], F32, tag="psw")
                    for kt in range(8):
                        nc.tensor.matmul(
                            pt[:], xt_sb[:, kt, 128 * st:128 * st + 128],
                            wv_sb[:, kt, 512 * hf:512 * hf + 512],
                            start=(kt == 0), stop=False)
                    nc.tensor.matmul(
                        pt[:], ones[0:1, 0:P],
                        bv_sb[0:1, 512 * hf:512 * hf + 512],
                        start=False, stop=True)
                    nc.vector.tensor_copy(
                        vt[:, 8 * hf:8 * hf + 8, 0:64],
                        pt[:].rearrange("p (h d) -> p h d", h=8))
                nc.vector.tensor_copy(vt[:, :, 64:65], ones[:, 0:16, None])
                nc.sync.dma_start(v_ap[:, st, :, :], vt[:])

            # ONE AllGather for K^T + V
            cc = nc.gpsimd.collective_compute(
                "AllGather", mybir.AluOpType.bypass, replica_groups=rg,
                ins=[kv_loc[:]], outs=[kv_all[:]])

            # Q (stays in SBUF, bf16) -- overlaps the collective
            wq_sb = load_w(wqT)

            def q_sink(dt, pt):
                nc.vector.tensor_copy(qt_sb[:, dt, :], pt[:])
            proj_T(wq_sb, bq_sb, q_sink)
            wo_sb = load_w(woT)

            # stage gathered K^T / V into SBUF
            kt_sb = kvp.tile([P, 4, 8, 512], BF)
            v_sb = kvp.tile([P, 16, 16, 65], BF)
            for r in range(4):
                src = kv_all[r, 0:KT_N].rearrange("(o p t) -> p o t",
                                                  o=8, p=P, t=512)
                dk = nc.sync.dma_start(kt_sb[:, r, :, :], src[:, :, :])
                add_dep_helper(dk.ins, cc.ins, reason="read after AG")
                vsrc = kv_all[r, KT_N:KV_N].rearrange(
                    "(a p h c) -> p a h c", a=4, p=P, h=16, c=65)
                for a in range(4):
                    dv = nc.sync.dma_start(v_sb[:, 4 * r + a, :, :],
                                           vsrc[:, a, :, :])
                    add_dep_helper(dv.ins, cc.ins, reason="read after AG")

            # ---------- phase 2: attention, head pairs ----------
            ctx_sb = big.tile([P, 8, 512], BF)
            for pair in range(8):
                h0, h1 = 2 * pair, 2 * pair + 1
                ctx0 = ps_ctx.tile([P, 512], F32, tag="ctx")
                ctx1 = ps_ctx.tile([P, 512], F32, tag="ctx")
                for j in range(16):
                    wid = 512 if (not causal or j < 8) else 256
                    qoff = 0 if (not causal or j < 8) else 256
                    rr, off = _kblk(j)
                    sc = ps_sc.tile([P, 1024], F32, tag="sc")
                    # two heads row-tiled: run concurrently on the PE
                    nc.tensor.matmul(
                        sc[:, 0:wid],
                        kt_sb[0:64, rr, pair, off:off + 128],
                        qt_sb[0:64, pair, qoff:qoff + wid],
                        start=True, stop=True)
                    nc.tensor.matmul(
                        sc[:, 512:512 + wid],
                        kt_sb[64:128, rr, pair, off:off + 128],
                        qt_sb[64:128, pair, qoff:qoff + wid],
                        start=True, stop=True)
                    et = ex.tile([P, 2, 512], BF, tag="exp")
                    if wid == 512:
                        nc.scalar.activation(et[:, :, :], sc[:, :], fexp)
                    else:
                        nc.scalar.activation(
                            et[:, :, 0:256],
                            sc[:].rearrange("p (s n) -> p s n", s=2)
                            [:, :, 0:256], fexp)
                    if causal:
                        nc.vector.tensor_tensor(
                            et[:, :, 0:256], et[:, :, 0:256],
                            cmb_sb[:, j, :, :], mybir.AluOpType.mult)
                    vj = 4 * rr + off // 128   # v_sb slot (rank-major order)
                    nc.tensor.matmul(
                        ctx0[0:65, qoff:qoff + wid],
                        v_sb[:, vj, h0, 0:65], et[:, 0, 0:wid],
                        start=(j == 0), stop=(j == 15))
                    nc.tensor.matmul(
                        ctx1[0:65, qoff:qoff + wid],
                        v_sb[:, vj, h1, 0:65], et[:, 1, 0:wid],
                        start=(j == 0), stop=(j == 15))
                # normalize: recip = exp(-ln(denominator)); even head lands
                # on partitions 0:64 of ctx_sb, odd head is moved to 64:128
                # with a small SBUF->SBUF DMA (cross-partition).
                for hi, ctxp in ((0, ctx0), (1, ctx1)):
                    lnd = sm.tile([1, 512], F32, tag="lnd")
                    nc.scalar.activation(lnd[:], ctxp[64:65, 0:512], fln)
                    rcp = sm.tile([1, 512], BF, tag="rcp")
                    nc.scalar.activation(rcp[:], lnd[:], fexp, scale=-1.0)
                    rep_ps = ps_w.tile([P, 512], F32, tag="psw")
                    nc.tensor.matmul(rep_ps[0:64, :], ones[0:1, 0:64],
                                     rcp[0:1, :], start=True, stop=True)
                    rep_sb = sm.tile([64, 512], F32, tag="rep")
                    nc.vector.tensor_copy(rep_sb[:], rep_ps[0:64, :])
                    if hi == 0:
                        nc.vector.tensor_tensor(
                            ctx_sb[0:64, pair, :], ctxp[0:64, :],
                            rep_sb[:], mybir.AluOpType.mult)
                    else:
                        ctmp = sm.tile([64, 512], BF, tag="ctmp")
                        nc.vector.tensor_tensor(
                            ctmp[:], ctxp[0:64, :], rep_sb[:],
                            mybir.AluOpType.mult)
                        nc.sync.dma_start(ctx_sb[64:128, pair, :], ctmp[:])

            # ---------- phase 3: output projection ----------
            for m in range(8):
                pt = ps_w.tile([P, 512], F32, tag="psw")
                for kt in range(8):
                    nc.tensor.matmul(
                        pt[:], wo_sb[:, kt, 128 * m:128 * m + 128],
                        ctx_sb[:, kt, :], start=(kt == 0), stop=False)
                nc.tensor.matmul(
                    pt[:], bo_sb[0:1, 128 * m:128 * m + 128],
                    ones[0:1, 0:512], start=False, stop=True)
                t = oio.tile([P, 512], F32, tag="oio")
                nc.vector.tensor_copy(t[:], pt[:])
                nc.sync.dma_start(
                    outT.rearrange("(o p) t -> p o t", p=P)[:, m, :], t[:])

    nc.compile()
    return nc


_CACHE = {}


def _get_nc(causal: bool, repeat: int = 1):
    key = (causal, repeat)
    if key not in _CACHE:
        _CACHE[key] = _emit(causal, repeat)
    return _CACHE[key]


def _mask01(p):
    """Per-core multiplicative mask [128, 16, 2, 256] for the causal
    staircase (same mask for both heads of a pair, hence the dim of 2)."""
    k = np.arange(128)[:, None]
    c = np.arange(256)[None, :]
    m1 = (c - k >= 0).astype(np.float32)
    m2 = (c - 128 - k >= 0).astype(np.float32)
    cmb = np.ones((128, 16, 256), dtype=np.float32)
    # j<8: masks the LOW chunk (cols 0:256 of the 512-wide tile)
    for j in range(8):
        if j == 2 * p:
            cmb[:, j, :] = m1
        elif j == 2 * p + 1:
            cmb[:, j, :] = m2
        elif j > 2 * p + 1:
            cmb[:, j, :] = 0.0
    # j>=8: masks the HIGH chunk (the only 256 cols computed)
    for j in range(8, 16):
        if j == 14 - 2 * p:
            cmb[:, j, :] = m1
        elif j == 15 - 2 * p:
            cmb[:, j, :] = m2
        elif j > 15 - 2 * p:
            cmb[:, j, :] = 0.0
    return np.ascontiguousarray(
        np.broadcast_to(cmb[:, :, None, :], (128, 16, 2, 256))
    ).astype(NPBF)


def kernel(**inputs):
    x = np.asarray(inputs["x"], dtype=np.float32)
    Wq = np.asarray(inputs["Wq"], dtype=np.float32)
    bq = np.asarray(inputs["bq"], dtype=np.float32)
    Wk = np.asarray(inputs["Wk"], dtype=np.float32)
    bk = np.asarray(inputs["bk"], dtype=np.float32)
    Wv = np.asarray(inputs["Wv"], dtype=np.float32)
    bv = np.asarray(inputs["bv"], dtype=np.float32)
    Wo = np.asarray(inputs["Wo"], dtype=np.float32)
    bo = np.asarray(inputs["bo"], dtype=np.float32)
    causal = bool(int(np.asarray(inputs["enable_causal"])))

    scale = np.float32(1.0 / np.sqrt(HD))
    wqT = np.ascontiguousarray(Wq.T).astype(NPBF)
    wkT = np.ascontiguousarray((Wk * scale).T).astype(NPBF)
    wvT = np.ascontiguousarray(Wv.T).astype(NPBF)
    woT = np.ascontiguousarray(Wo.T).astype(NPBF)
    bqr = bq.reshape(1, D).astype(NPBF)
    bkr = (bk * scale).reshape(1, D).astype(NPBF)
    bvr = bv.reshape(1, D).astype(NPBF)
    bor = bo.reshape(1, D).astype(NPBF)

    nc = _get_nc(causal)
    in_maps = []
    for c in range(NC):
        b, p = divmod(c, 4)
        sel = sel_tokens(p)
        xTc = np.ascontiguousarray(x[b][sel, :].T).astype(NPBF)
        m = {"xT": xTc, "wqT": wqT, "wkT": wkT, "wvT": wvT, "woT": woT,
             "bq": bqr, "bk": bkr, "bv": bvr, "bo": bor}
        if causal:
            m["cmb"] = _mask01(p)
        in_maps.append(m)

    global LAST_RESULT
    res = run_bass_kernel_spmd(nc, in_maps, list(range(NC)), trace=TRACE)
    LAST_RESULT = res
    out = np.empty((B, S, D), dtype=np.float32)
    for c in range(NC):
        b, p = divmod(c, 4)
        sel = sel_tokens(p)
        out[b, sel, :] = np.asarray(res.results[c]["outT"], dtype=np.float32).T
    return out


# revision 7
# speedup vs baseline: 1.8918x; 1.0185x over previous
"""Trainium2 Bass kernel for nn_MultiHeadAttention (B=2, S=2048, D=1024, H=16, causal).

Sharding across 8 NeuronCores (single SPMD program):
  - Core c owns batch b=c//4 and two 256-token query chunks {p, 7-p} (p=c%4);
    the pairing balances causal attention work.
  - Everything on-chip is bf16 (PSUM accumulation stays fp32): halves HBM +
    collective bytes vs fp32 and enables fast weight loads (FWL) on the PE.
  - Phase 1: project K^T, V (with softmax scale folded into Wk/bk on the
    host), publish both with ONE AllGather (replica groups [[0-3],[4-7]]);
    Q projection + Wo load overlap the collective.  All biases are applied
    with rank-1 matmuls into PSUM (no scalar-engine bias pass).
  - Phase 2: K^T and V for the whole batch live in SBUF.  Heads are
    processed in pairs (feature block = 128 partitions); per key block the
    two heads' score matmuls are row-tiled (partitions 0:64 / 64:128) into
    one 2-bank PSUM tile so they run concurrently, one wide exp covers both,
    causal masking is a single multiplicative bf16 DVE op on the (host
    per-core) staircase, and ctx accumulates in PSUM across ALL 16 key
    blocks (65th stationary column = softmax denominator).  The softmax
    reciprocal is exp(-ln(d)) on the scalar engine (DVE reciprocal is
    8 cyc/elem and was 67us in the fp32 baseline).
  - Phase 3: output projection for the core's own tokens (row-parallel over
    tokens => no reduction); host re-assembles the full output.
"""
import numpy as np
import ml_dtypes

import concourse.bass as bass
import concourse.bacc as bacc
import concourse.mybir as mybir
import concourse.tile as tile
from concourse.bass_utils import run_bass_kernel_spmd
from concourse.tile_rust import add_dep_helper

B, S, D, H, HD = 2, 2048, 1024, 16, 64
NC = 8
P = 128
F32 = mybir.dt.float32
BF = mybir.dt.bfloat16
NPBF = ml_dtypes.bfloat16

KT_N = D * 512           # K^T shard elems  [1024, 512]
V_N = 512 * 16 * 65      # V shard elems    [512 tok, 16 heads, 64+1]
KV_N = KT_N + V_N

TRACE = False        # set True (e.g. from test.py) to capture an NTFF profile
LAST_RESULT = None   # BassKernelResults of the most recent kernel() call

_ACT_PATCHED = False


def _patch_act_tables():
    """Steer Bacc's act-table-load pass to the combined natural_log+exp
    set.  The pass assigns each activation function the FIRST table set
    containing it, so a kernel using both Exp and Ln alternates between
    `exp_and_others` and `natural_log` -- one ~1.3us ACT_TABLE_LOAD per
    transition (17 loads / 22us on the scalar engine for this kernel).
    Hiding Exp/Ln from the earlier sets makes both resolve to the single
    `natural_log_exp_and_others` set (one load total).  List length and
    order are preserved, so the set ids walrus emits stay valid."""
    global _ACT_PATCHED
    if _ACT_PATCHED:
        return
    import concourse.bacc as _bacc
    _orig = _bacc.get_activation_tables

    def _filtered(arch):
        t = _orig(arch)
        fexp = mybir.ActivationFunctionType.Exp
        fln = mybir.ActivationFunctionType.Ln
        out = {}
        for name, fns in t.items():
            if name != "natural_log_exp_and_others" and (
                    fexp in fns or fln in fns):
                fns = fns - {fexp, fln}
            out[name] = fns
        return out

    _bacc.get_activation_tables = _filtered
    _ACT_PATCHED = True


def sel_tokens(p):
    return list(range(256 * p, 256 * p + 256)) + list(
        range(256 * (7 - p), 256 * (7 - p) + 256)
    )


def _kblk(j):
    """Original 128-token key block j -> (rank-in-group, column offset)."""
    q = j // 2
    rr = q if q <= 3 else 7 - q
    off = (0 if q <= 3 else 256) + 128 * (j % 2)
    return rr, off


def _emit(causal: bool, repeat: int = 1):
    nc = bacc.Bacc(trn_type="TRN2", num_devices=NC)
    fexp = mybir.ActivationFunctionType.Exp
    fln = mybir.ActivationFunctionType.Ln
    _patch_act_tables()

    xT = nc.dram_tensor("xT", [D, 512], BF, kind="ExternalInput")
    wqT = nc.dram_tensor("wqT", [D, D], BF, kind="ExternalInput")
    wkT = nc.dram_tensor("wkT", [D, D], BF, kind="ExternalInput")
    wvT = nc.dram_tensor("wvT", [D, D], BF, kind="ExternalInput")
    woT = nc.dram_tensor("woT", [D, D], BF, kind="ExternalInput")
    bq_d = nc.dram_tensor("bq", [1, D], BF, kind="ExternalInput")
    bk_d = nc.dram_tensor("bk", [1, D], BF, kind="ExternalInput")
    bv_d = nc.dram_tensor("bv", [1, D], BF, kind="ExternalInput")
    bo_d = nc.dram_tensor("bo", [1, D], BF, kind="ExternalInput")
    if causal:
        cmb_d = nc.dram_tensor("cmb", [P, 16, 2, 256], BF, kind="ExternalInput")
    outT = nc.dram_tensor("outT", [D, 512], F32, kind="ExternalOutput")

    kv_loc = nc.dram_tensor("kv_loc", [KV_N], BF)
    kv_all = nc.dram_tensor("kv_all", [4, KV_N], BF)

    with tile.TileContext(nc) as tc, \
         tc.tile_pool(name="const", bufs=1) as const, \
         tc.tile_pool(name="w", bufs=2) as wpool, \
         tc.tile_pool(name="big", bufs=1) as big, \
         tc.tile_pool(name="kv", bufs=1) as kvp, \
         tc.tile_pool(name="io", bufs=3) as io, \
         tc.tile_pool(name="vio", bufs=2) as vio, \
         tc.tile_pool(name="oio", bufs=2) as oio, \
         tc.tile_pool(name="ex", bufs=3) as ex, \
         tc.tile_pool(name="sm", bufs=2) as sm, \
         tc.tile_pool(name="ps_sc", bufs=2, space="PSUM") as ps_sc, \
         tc.tile_pool(name="ps_ctx", bufs=2, space="PSUM") as ps_ctx, \
         tc.tile_pool(name="ps_w", bufs=2, space="PSUM") as ps_w:

        # ---------- constants ----------
        ones = const.tile([P, 512], BF)
        nc.gpsimd.memset(ones[:], 1.0)
        bq_sb = const.tile([1, D], BF)
        nc.sync.dma_start(bq_sb[:], bq_d[:])
        bk_sb = const.tile([1, D], BF)
        nc.sync.dma_start(bk_sb[:], bk_d[:])
        bv_sb = const.tile([1, D], BF)
        nc.sync.dma_start(bv_sb[:], bv_d[:])
        bo_sb = const.tile([1, D], BF)
        nc.sync.dma_start(bo_sb[:], bo_d[:])
        if causal:
            cmb_sb = big.tile([P, 16, 2, 256], BF)
            nc.sync.dma_start(cmb_sb[:], cmb_d[:])

        rg = [[0, 1, 2, 3], [4, 5, 6, 7]]
        kt_ap = kv_loc[0:KT_N].rearrange("(o p t) -> p o t", o=8, p=P, t=512)
        v_ap = kv_loc[KT_N:KV_N].rearrange("(a p h c) -> p a h c",
                                           a=4, p=P, h=16, c=65)

        for _rep in range(repeat):
            # ---------- phase 1: projections for this core's 512 tokens ----
            xt_sb = big.tile([P, 8, 512], BF)
            xr = xT.rearrange("(o p) t -> p o t", p=P)
            for _kt in range(8):
                nc.sync.dma_start(xt_sb[:, _kt, :], xr[:, _kt, :])
            qt_sb = big.tile([P, 8, 512], BF)

            def load_w(w_dram):
                w_sb = wpool.tile([P, 8, D], BF, tag="w")
                wr = w_dram.rearrange("(o p) t -> p o t", p=P)
                for _kt in range(8):
                    nc.sync.dma_start(w_sb[:, _kt, :], wr[:, _kt, :])
                return w_sb

            def proj_T(w_sb, bias_sb, sink):
                # out[feat, tok]: per-partition bias via rank-1 matmul
                for dt in range(8):
                    pt = ps_w.tile([P, 512], F32, tag="psw")
                    for kt in range(8):
                        nc.tensor.matmul(
                            pt[:], w_sb[:, kt, 128 * dt:128 * dt + 128],
                            xt_sb[:, kt, :], start=(kt == 0), stop=False)
                    nc.tensor.matmul(
                        pt[:], bias_sb[0:1, 128 * dt:128 * dt + 128],
                        ones[0:1, 0:512], start=False, stop=True)
                    sink(dt, pt)

            # K^T -> kv_loc
            wk_sb = load_w(wkT)

            def k_sink(dt, pt):
                t = io.tile([P, 512], BF, tag="io")
                nc.vector.tensor_copy(t[:], pt[:])
                nc.sync.dma_start(kt_ap[:, dt, :], t[:])
            proj_T(wk_sb, bk_sb, k_sink)

            # V -> kv_loc ([tok, head, 64] + ones column)
            wv_sb = load_w(wvT)
            for st in range(4):
                vt = vio.tile([P, 16, 65], BF, tag="vio")
                for hf in range(2):
                    pt = ps_w.tile([P, 512], F32, tag="psw")
                    for kt in range(8):
                        nc.tensor.matmul(
                            pt[:], xt_sb[:, kt, 128 * st:128 * st + 128],
                            wv_sb[:, kt, 512 * hf:512 * hf + 512],
                            start=(kt == 0), stop=False)
                    nc.tensor.matmul(
                        pt[:], ones[0:1, 0:P],
                        bv_sb[0:1, 512 * hf:512 * hf + 512],
                        start=False, stop=True)
                    nc.vector.tensor_copy(
                        vt[:, 8 * hf:8 * hf + 8, 0:64],
                        pt[:].rearrange("p (h d) -> p h d", h=8))
                nc.vector.tensor_copy(vt[:, :, 64:65], ones[:, 0:16, None])
                nc.sync.dma_start(v_ap[:, st, :, :], vt[:])

            # ONE AllGather for K^T + V
            cc = nc.gpsimd.collective_compute(
                "AllGather", mybir.AluOpType.bypass, replica_groups=rg,
                ins=[kv_loc[:]], outs=[kv_all[:]])

            # Q (stays in SBUF, bf16) -- overlaps the collective
            wq_sb = load_w(wqT)

            def q_sink(dt, pt):
                nc.vector.tensor_copy(qt_sb[:, dt, :], pt[:])
            proj_T(wq_sb, bq_sb, q_sink)
            wo_sb = load_w(woT)

            # stage gathered K^T / V into SBUF
            kt_sb = kvp.tile([P, 4, 8, 512], BF)
            v_sb = kvp.tile([P, 16, 16, 65], BF)
            for r in range(4):
                src = kv_all[r, 0:KT_N].rearrange("(o p t) -> p o t",
                                                  o=8, p=P, t=512)
                dk = nc.sync.dma_start(kt_sb[:, r, :, :], src[:, :, :])
                add_dep_helper(dk.ins, cc.ins, reason="read after AG")
                vsrc = kv_all[r, KT_N:KV_N].rearrange(
                    "(a p h c) -> p a h c", a=4, p=P, h=16, c=65)
                for a in range(4):
                    dv = nc.sync.dma_start(v_sb[:, 4 * r + a, :, :],
                                           vsrc[:, a, :, :])
                    add_dep_helper(dv.ins, cc.ins, reason="read after AG")

            # ---------- phase 2: attention, head pairs ----------
            ctx_sb = big.tile([P, 8, 512], BF)
            for pair in range(8):
                h0, h1 = 2 * pair, 2 * pair + 1
                ctx0 = ps_ctx.tile([P, 512], F32, tag="ctx")
                ctx1 = ps_ctx.tile([P, 512], F32, tag="ctx")
                for j in range(16):
                    wid = 512 if (not causal or j < 8) else 256
                    qoff = 0 if (not causal or j < 8) else 256
                    rr, off = _kblk(j)
                    sc = ps_sc.tile([P, 1024], F32, tag="sc")
                    # two heads row-tiled: run concurrently on the PE
                    nc.tensor.matmul(
                        sc[:, 0:wid],
                        kt_sb[0:64, rr, pair, off:off + 128],
                        qt_sb[0:64, pair, qoff:qoff + wid],
                        start=True, stop=True)
                    nc.tensor.matmul(
                        sc[:, 512:512 + wid],
                        kt_sb[64:128, rr, pair, off:off + 128],
                        qt_sb[64:128, pair, qoff:qoff + wid],
                        start=True, stop=True)
                    et = ex.tile([P, 2, 512], BF, tag="exp")
                    if wid == 512:
                        nc.scalar.activation(et[:, :, :], sc[:, :], fexp)
                    else:
                        nc.scalar.activation(
                            et[:, :, 0:256],
                            sc[:].rearrange("p (s n) -> p s n", s=2)
                            [:, :, 0:256], fexp)
                    if causal:
                        nc.vector.tensor_tensor(
                            et[:, :, 0:256], et[:, :, 0:256],
                            cmb_sb[:, j, :, :], mybir.AluOpType.mult)
                    vj = 4 * rr + off // 128   # v_sb slot (rank-major order)
                    nc.tensor.matmul(
                        ctx0[0:65, qoff:qoff + wid],
                        v_sb[:, vj, h0, 0:65], et[:, 0, 0:wid],
                        start=(j == 0), stop=(j == 15))
                    nc.tensor.matmul(
                        ctx1[0:65, qoff:qoff + wid],
                        v_sb[:, vj, h1, 0:65], et[:, 1, 0:wid],
                        start=(j == 0), stop=(j == 15))
                # normalize: recip = exp(-ln(denominator)); even head lands
                # on partitions 0:64 of ctx_sb, odd head is moved to 64:128
                # with a small SBUF->SBUF DMA (cross-partition).
                for hi, ctxp in ((0, ctx0), (1, ctx1)):
                    lnd = sm.tile([1, 512], F32, tag="lnd")
                    nc.scalar.activation(lnd[:], ctxp[64:65, 0:512], fln)
                    rcp = sm.tile([1, 512], BF, tag="rcp")
                    nc.scalar.activation(rcp[:], lnd[:], fexp, scale=-1.0)
                    rep_ps = ps_w.tile([P, 512], F32, tag="psw")
                    nc.tensor.matmul(rep_ps[0:64, :], ones[0:1, 0:64],
                                     rcp[0:1, :], start=True, stop=True)
                    rep_sb = sm.tile([64, 512], F32, tag="rep")
                    nc.vector.tensor_copy(rep_sb[:], rep_ps[0:64, :])
                    if hi == 0:
                        nc.vector.tensor_tensor(
                            ctx_sb[0:64, pair, :], ctxp[0:64, :],
                            rep_sb[:], mybir.AluOpType.mult)
                    else:
                        ctmp = sm.tile([64, 512], BF, tag="ctmp")
                        nc.vector.tensor_tensor(
                            ctmp[:], ctxp[0:64, :], rep_sb[:],
                            mybir.AluOpType.mult)
                        nc.sync.dma_start(ctx_sb[64:128, pair, :], ctmp[:])

            # ---------- phase 3: output projection ----------
            for m in range(8):
                pt = ps_w.tile([P, 512], F32, tag="psw")
                for kt in range(8):
                    nc.tensor.matmul(
                        pt[:], wo_sb[:, kt, 128 * m:128 * m + 128],
                        ctx_sb[:, kt, :], start=(kt == 0), stop=False)
                nc.tensor.matmul(
                    pt[:], bo_sb[0:1, 128 * m:128 * m + 128],
                    ones[0:1, 0:512], start=False, stop=True)
                t = oio.tile([P, 512], F32, tag="oio")
                nc.vector.tensor_copy(t[:], pt[:])
                nc.sync.dma_start(
                    outT.rearrange("(o p) t -> p o t", p=P)[:, m, :], t[:])

    nc.compile()
    return nc


_CACHE = {}


def _get_nc(causal: bool, repeat: int = 1):
    key = (causal, repeat)
    if key not in _CACHE:
        _CACHE[key] = _emit(causal, repeat)
    return _CACHE[key]


def _mask01(p):
    """Per-core multiplicative mask [128, 16, 2, 256] for the causal
    staircase (same mask for both heads of a pair, hence the dim of 2)."""
    k = np.arange(128)[:, None]
    c = np.arange(256)[None, :]
    m1 = (c - k >= 0).astype(np.float32)
    m2 = (c - 128 - k >= 0).astype(np.float32)
    cmb = np.ones((128, 16, 256), dtype=np.float32)
    # j<8: masks the LOW chunk (cols 0:256 of the 512-wide tile)
    for j in range(8):
        if j == 2 * p:
            cmb[:, j, :] = m1
        elif j == 2 * p + 1:
            cmb[:, j, :] = m2
        elif j > 2 * p + 1:
            cmb[:, j, :] = 0.0
    # j>=8: masks the HIGH chunk (the only 256 cols computed)
    for j in range(8, 16):
        if j == 14 - 2 * p:
            cmb[:, j, :] = m1
        elif j == 15 - 2 * p:
            cmb[:, j, :] = m2
        elif j > 15 - 2 * p:
            cmb[:, j, :] = 0.0
    return np.ascontiguousarray(
        np.broadcast_to(cmb[:, :, None, :], (128, 16, 2, 256))
    ).astype(NPBF)


def kernel(**inputs):
    x = np.asarray(inputs["x"], dtype=np.float32)
    Wq = np.asarray(inputs["Wq"], dtype=np.float32)
    bq = np.asarray(inputs["bq"], dtype=np.float32)
    Wk = np.asarray(inputs["Wk"], dtype=np.float32)
    bk = np.asarray(inputs["bk"], dtype=np.float32)
    Wv = np.asarray(inputs["Wv"], dtype=np.float32)
    bv = np.asarray(inputs["bv"], dtype=np.float32)
    Wo = np.asarray(inputs["Wo"], dtype=np.float32)
    bo = np.asarray(inputs["bo"], dtype=np.float32)
    causal = bool(int(np.asarray(inputs["enable_causal"])))

    scale = np.float32(1.0 / np.sqrt(HD))
    wqT = np.ascontiguousarray(Wq.T).astype(NPBF)
    wkT = np.ascontiguousarray((Wk * scale).T).astype(NPBF)
    wvT = np.ascontiguousarray(Wv.T).astype(NPBF)
    woT = np.ascontiguousarray(Wo.T).astype(NPBF)
    bqr = bq.reshape(1, D).astype(NPBF)
    bkr = (bk * scale).reshape(1, D).astype(NPBF)
    bvr = bv.reshape(1, D).astype(NPBF)
    bor = bo.reshape(1, D).astype(NPBF)

    nc = _get_nc(causal)
    in_maps = []
    for c in range(NC):
        b, p = divmod(c, 4)
        sel = sel_tokens(p)
        xTc = np.ascontiguousarray(x[b][sel, :].T).astype(NPBF)
        m = {"xT": xTc, "wqT": wqT, "wkT": wkT, "wvT": wvT, "woT": woT,
             "bq": bqr, "bk": bkr, "bv": bvr, "bo": bor}
        if causal:
            m["cmb"] = _mask01(p)
        in_maps.append(m)

    global LAST_RESULT
    res = run_bass_kernel_spmd(nc, in_maps, list(range(NC)), trace=TRACE)
    LAST_RESULT = res
    out = np.empty((B, S, D), dtype=np.float32)
    for c in range(NC):
        b, p = divmod(c, 4)
        sel = sel_tokens(p)
        out[b, sel, :] = np.asarray(res.results[c]["outT"], dtype=np.float32).T
    return out
